# revision 1
# baseline (speedup 1.0000x reference)
"""Cayley orthogonal transform kernel for Trainium2 (8 NeuronCores).

Math: per head h, y = (I - S) ((1+eps) I + S)^{-1} x applied along D=128,
where S = S_raw - S_raw^T is skew-symmetric.

Strategy:
  * Host: skew-symmetrize S_raw, and lay x out as xT[h, d, token] (token-major
    per head) so the device only ever runs plain matmuls - no on-device
    transposes.  Heads are sharded 2-per-core across the 8 cores (tensor
    parallel, embarrassingly parallel per the problem structure).
  * Device (per core): build W^T = ((1+eps)I - S)^{-1} (I + S) per head with a
    Newton-Schulz iteration (pure 128x128 matmuls, converges to fp32 accuracy
    in <=7 iterations since ||S||_2 ~ 1.6), then stream the (128 x 16384)
    token panel through the PE array in 512-column fp32 matmuls:
        yT[h] = W @ xT[h]
    PSUM results are evacuated to SBUF alternating Vector/Scalar engines and
    DMA'd back to DRAM in 2 MiB chunks.  The kernel is HBM-bandwidth bound
    (~34 MB of DRAM traffic per core); all compute hides under the DMA.
  * Host: inverse layout transform back to (B, H, N, D).
"""

import os
import sys

import numpy as np

B, H, N, D = 4, 16, 4096, 128
N_CORES = 8
HPC = H // N_CORES          # heads per core
T = B * N                   # tokens per head
CHUNK = 4096                # columns per DMA tile (2 MiB)
MM = 512                    # columns per fp32 matmul (one PSUM bank)
NS_ITERS = 7                # Newton-Schulz iterations
NS_C = 0.42                 # NS initial scale: X0 = c * G^T  (safe for ||S||<~1.9)
EPS = 1e-5

_CACHE = {}


def _ensure_path():
    for p in ("/opt/trn_rl_repo", "/root/.axon_site/_ro/trn_rl_repo"):
        if os.path.isdir(p) and p not in sys.path:
            sys.path.insert(0, p)
    _install_ntff_hook()


def _install_ntff_hook():
    """The agent image's ``antenv`` lacks ``axon_hooks``, which makes
    ``run_bass_kernel_spmd(trace=True)`` crash instead of degrading.  Provide
    the module and register the ctypes NTFF hook the boot shim would have."""
    if "antenv.axon_hooks" in sys.modules:
        return
    try:
        import types

        import antenv

        if hasattr(antenv, "axon_hooks"):
            return
        mod = types.ModuleType("antenv.axon_hooks")
        state = {"hook": None}
        mod.set_axon_ntff_profile_hook = lambda h: state.__setitem__("hook", h)
        mod.get_axon_ntff_profile_hook = lambda: state["hook"]
        sys.modules["antenv.axon_hooks"] = mod
        antenv.axon_hooks = mod
        try:
            from trn_agent_boot.trn_boot import _ntff_profile_via_ctypes

            so_path = "/opt/axon/libaxon_pjrt.so"
            if os.path.exists(so_path):
                mod.set_axon_ntff_profile_hook(_ntff_profile_via_ctypes(so_path))
        except Exception:
            pass  # hook stays None -> concourse logs + skips tracing
    except Exception:
        pass


def _build_nc():
    """Build the (single-program SPMD) Bass kernel for one core's shard."""
    _ensure_path()
    import concourse.tile as tile
    from concourse import bacc, mybir
    from concourse.masks import make_identity

    f32 = mybir.dt.float32
    bf16 = mybir.dt.bfloat16
    Alu = mybir.AluOpType

    nc = bacc.Bacc("TRN2", target_bir_lowering=False, debug=False)
    xhi_d = nc.dram_tensor("xhi", [HPC * D, T], bf16, kind="ExternalInput").ap()
    xlo_d = nc.dram_tensor("xlo", [HPC * D, T], bf16, kind="ExternalInput").ap()
    s_d = nc.dram_tensor("s", [HPC * D, D], f32, kind="ExternalInput").ap()
    yT_d = nc.dram_tensor("yT", [HPC * D, T], f32, kind="ExternalOutput").ap()

    with tile.TileContext(nc) as tc:
        with (
            tc.tile_pool(name="const", bufs=1) as const_pool,
            tc.tile_pool(name="ns", bufs=2) as ns_pool,
            tc.tile_pool(name="wt", bufs=1) as wt_pool,
            tc.tile_pool(name="xin", bufs=6) as in_pool,
            tc.tile_pool(name="yout", bufs=3) as out_pool,
            tc.tile_pool(name="mmps", bufs=4, space="PSUM") as ps_big,
            tc.tile_pool(name="nsps", bufs=4, space="PSUM") as ps_ns,
        ):
            ident = const_pool.tile([D, D], f32, tag="ident")
            make_identity(nc, ident)
            twoE = const_pool.tile([D, D], f32, tag="twoE")
            nc.vector.tensor_scalar_mul(twoE, ident, 2.0)

            # --- Newton-Schulz per head: WT = Ginv @ (I + S), G = (1+eps)I - S
            # bass matmul computes lhsT.T @ rhs; note A := (1+eps)I + S = G^T.
            wts = []
            for h in range(HPC):
                s_sb = const_pool.tile([D, D], f32, tag=f"s{h}")
                nc.sync.dma_start(out=s_sb, in_=s_d[h * D:(h + 1) * D, :])
                a_mat = const_pool.tile([D, D], f32, tag=f"amat{h}")
                nc.vector.scalar_tensor_tensor(
                    out=a_mat, in0=ident, scalar=1.0 + EPS, in1=s_sb,
                    op0=Alu.mult, op1=Alu.add)
                ips = const_pool.tile([D, D], f32, tag=f"ips{h}")
                nc.vector.tensor_add(ips, ident, s_sb)
                g_mat = const_pool.tile([D, D], f32, tag=f"g{h}")
                nc.vector.scalar_tensor_tensor(
                    out=g_mat, in0=ident, scalar=1.0 + EPS, in1=s_sb,
                    op0=Alu.mult, op1=Alu.subtract)

                X = ns_pool.tile([D, D], f32, tag=f"x{h}")
                nc.vector.tensor_scalar_mul(X, a_mat, NS_C)    # X0 = c G^T
                XT = ns_pool.tile([D, D], f32, tag=f"xt{h}")
                nc.vector.tensor_scalar_mul(XT, g_mat, NS_C)   # X0^T = c G

                for k in range(NS_ITERS):
                    t_ps = ps_ns.tile([D, D], f32, tag="nsps")
                    nc.tensor.matmul(t_ps, lhsT=a_mat, rhs=X, start=True, stop=True)  # G X
                    t2 = ns_pool.tile([D, D], f32, tag=f"t2{h}")
                    nc.vector.tensor_sub(t2, twoE, t_ps)       # 2I - G X
                    if k < NS_ITERS - 1:
                        xn_ps = ps_ns.tile([D, D], f32, tag="nsps")
                        nc.tensor.matmul(xn_ps, lhsT=XT, rhs=t2, start=True, stop=True)  # X T2
                        Xn = ns_pool.tile([D, D], f32, tag=f"x{h}")
                        nc.scalar.copy(Xn, xn_ps)
                        X = Xn
                    xtn_ps = ps_ns.tile([D, D], f32, tag="nsps")
                    nc.tensor.matmul(xtn_ps, lhsT=t2, rhs=XT, start=True, stop=True)  # (X T2)^T
                    XTn = ns_pool.tile([D, D], f32, tag=f"xt{h}")
                    nc.scalar.copy(XTn, xtn_ps)
                    XT = XTn

                wt_ps = ps_ns.tile([D, D], f32, tag="nsps")
                nc.tensor.matmul(wt_ps, lhsT=XT, rhs=ips, start=True, stop=True)  # Ginv (I+S)
                # split W^T into bf16 hi+lo for the 3-term bf16 panel matmul
                whi = wt_pool.tile([D, D], bf16, tag=f"whi{h}")
                nc.vector.tensor_copy(whi, wt_ps)
                wlo = wt_pool.tile([D, D], bf16, tag=f"wlo{h}")
                nc.vector.tensor_sub(wlo, wt_ps, whi)
                wts.append((whi, wlo))

            # --- streaming panel matmul: yT[h] = W @ xT[h]
            # 3-term bf16 split (y = Whi(xhi+xlo) + Wlo xhi, fp32 PSUM accum):
            # bf16 runs the PE at 1 cycle/row (vs 4 for fp32) with fp32-class
            # accuracy (~4e-6), and hi+lo bf16 totals the same DMA bytes as
            # fp32 -- the kernel stays at the HBM roofline.
            half = CHUNK // 2
            for h in range(HPC):
                whi, wlo = wts[h]
                r0 = h * D
                for ci in range(T // CHUNK):
                    c0 = ci * CHUNK
                    xh = in_pool.tile([D, CHUNK], bf16, tag="xh")
                    nc.sync.dma_start(out=xh, in_=xhi_d[r0:r0 + D, c0:c0 + CHUNK])
                    xl = in_pool.tile([D, CHUNK], bf16, tag="xl")
                    nc.sync.dma_start(out=xl, in_=xlo_d[r0:r0 + D, c0:c0 + CHUNK])
                    yout = out_pool.tile([D, CHUNK], f32, tag="yout")
                    for j in range(CHUNK // MM):
                        sl = slice(j * MM, (j + 1) * MM)
                        ps = ps_big.tile([D, MM], f32, tag="mm")
                        nc.tensor.matmul(ps, lhsT=whi, rhs=xh[:, sl],
                                         start=True, stop=False)
                        nc.tensor.matmul(ps, lhsT=whi, rhs=xl[:, sl],
                                         start=False, stop=False)
                        nc.tensor.matmul(ps, lhsT=wlo, rhs=xh[:, sl],
                                         start=False, stop=True)
                        if j % 2 == 0:
                            nc.vector.tensor_copy(yout[:, sl], ps)
                        else:
                            nc.scalar.copy(yout[:, sl], ps)
                    # two half-stores so the DMA overlaps the second half's evac
                    nc.scalar.dma_start(out=yT_d[r0:r0 + D, c0:c0 + half],
                                        in_=yout[:, 0:half])
                    nc.scalar.dma_start(out=yT_d[r0:r0 + D, c0 + half:c0 + CHUNK],
                                        in_=yout[:, half:CHUNK])
    nc.compile()
    return nc


def _get_nc():
    if "nc" not in _CACHE:
        _CACHE["nc"] = _build_nc()
    return _CACHE["nc"]


def _prep_inputs(x, S_raw):
    """Host-side shard + layout prep. Returns per-core input maps."""
    import ml_dtypes

    bf16 = ml_dtypes.bfloat16
    x = np.asarray(x, dtype=np.float32)
    S_raw = np.asarray(S_raw, dtype=np.float32)
    S = S_raw - S_raw.transpose(0, 2, 1)
    # (B,H,N,D) -> (H, D, B*N), token-major per head; bf16 hi/lo split
    xT_full = np.ascontiguousarray(x.transpose(1, 3, 0, 2)).reshape(H * D, T)
    xhi = xT_full.astype(bf16)
    xlo = (xT_full - xhi.astype(np.float32)).astype(bf16)
    S_full = np.ascontiguousarray(S).reshape(H * D, D)
    in_maps = []
    for c in range(N_CORES):
        r = c * HPC * D
        in_maps.append({
            "xhi": xhi[r:r + HPC * D],
            "xlo": xlo[r:r + HPC * D],
            "s": S_full[r:r + HPC * D],
        })
    return in_maps


def _postprocess(results):
    """Gather per-core yT shards back into (B, H, N, D)."""
    yT_full = np.concatenate([r["yT"] for r in results], axis=0)  # (H*D, T)
    y = yT_full.reshape(H, D, B, N).transpose(2, 0, 3, 1)
    return np.ascontiguousarray(y)


def _execute(in_maps, trace=False, **kwargs):
    _ensure_path()
    from concourse.bass_utils import run_bass_kernel_spmd

    nc = _get_nc()
    return run_bass_kernel_spmd(nc, in_maps, core_ids=list(range(N_CORES)),
                                trace=trace, **kwargs)


def kernel(x, S_raw):
    in_maps = _prep_inputs(x, S_raw)
    res = _execute(in_maps)
    return _postprocess(res.results)



# revision 2
# speedup vs baseline: 1.3749x; 1.3749x over previous
"""Cayley orthogonal transform kernel for Trainium2 (8 NeuronCores).

Math: per head h, y = (I - S) ((1+eps) I + S)^{-1} x applied along D=128,
where S = S_raw - S_raw^T is skew-symmetric.

Strategy:
  * Host: skew-symmetrize S_raw, and lay x out as xT[h, d, token] (token-major
    per head) so the device only ever runs plain matmuls - no on-device
    transposes.  Heads are sharded 2-per-core across the 8 cores (tensor
    parallel, embarrassingly parallel per the problem structure).
  * Device (per core): build W^T = ((1+eps)I - S)^{-1} (I + S) per head with a
    Newton-Schulz iteration (pure 128x128 matmuls, converges to fp32 accuracy
    in <=7 iterations since ||S||_2 ~ 1.6), then stream the (128 x 16384)
    token panel through the PE array in 512-column bf16 matmuls:
        yT[h] = W @ xT[h]
    PSUM results are evacuated to SBUF (cast to bf16) alternating Vector/
    Scalar engines and DMA'd back to DRAM.  Both x and y travel as single
    bf16 tensors (~4e-3 rel error, well within tolerance), so the kernel
    moves ~16.8 MB per core and sits at the per-core HBM roofline
    (~358 GB/s); all compute hides under the DMA.
  * Host: upcast to fp32 + inverse layout transform back to (B, H, N, D).
"""

import os
import sys

import numpy as np

B, H, N, D = 4, 16, 4096, 128
N_CORES = 8
HPC = H // N_CORES          # heads per core
T = B * N                   # tokens per head
CHUNK = 4096                # columns per DMA tile (1 MiB bf16)
MM = 512                    # columns per matmul (one PSUM bank)
NS_ITERS = 7                # Newton-Schulz iterations
NS_C = 0.42                 # NS initial scale: X0 = c * G^T  (safe for ||S||<~1.9)
EPS = 1e-5

_CACHE = {}


def _ensure_path():
    for p in ("/opt/trn_rl_repo", "/root/.axon_site/_ro/trn_rl_repo"):
        if os.path.isdir(p) and p not in sys.path:
            sys.path.insert(0, p)
    _install_ntff_hook()


def _install_ntff_hook():
    """The agent image's ``antenv`` lacks ``axon_hooks``, which makes
    ``run_bass_kernel_spmd(trace=True)`` crash instead of degrading.  Provide
    the module and register the ctypes NTFF hook the boot shim would have."""
    if "antenv.axon_hooks" in sys.modules:
        return
    try:
        import types

        import antenv

        if hasattr(antenv, "axon_hooks"):
            return
        mod = types.ModuleType("antenv.axon_hooks")
        state = {"hook": None}
        mod.set_axon_ntff_profile_hook = lambda h: state.__setitem__("hook", h)
        mod.get_axon_ntff_profile_hook = lambda: state["hook"]
        sys.modules["antenv.axon_hooks"] = mod
        antenv.axon_hooks = mod
        try:
            from trn_agent_boot.trn_boot import _ntff_profile_via_ctypes

            so_path = "/opt/axon/libaxon_pjrt.so"
            if os.path.exists(so_path):
                mod.set_axon_ntff_profile_hook(_ntff_profile_via_ctypes(so_path))
        except Exception:
            pass  # hook stays None -> concourse logs + skips tracing
    except Exception:
        pass


def _build_nc():
    """Build the (single-program SPMD) Bass kernel for one core's shard."""
    _ensure_path()
    import concourse.tile as tile
    from concourse import bacc, mybir
    from concourse.masks import make_identity

    f32 = mybir.dt.float32
    bf16 = mybir.dt.bfloat16
    Alu = mybir.AluOpType

    nc = bacc.Bacc("TRN2", target_bir_lowering=False, debug=False)
    x_d = nc.dram_tensor("x", [HPC * D, T], bf16, kind="ExternalInput").ap()
    s_d = nc.dram_tensor("s", [HPC * D, D], f32, kind="ExternalInput").ap()
    yT_d = nc.dram_tensor("yT", [HPC * D, T], bf16, kind="ExternalOutput").ap()

    with tile.TileContext(nc) as tc:
        with (
            tc.tile_pool(name="const", bufs=1) as const_pool,
            tc.tile_pool(name="ns", bufs=2) as ns_pool,
            tc.tile_pool(name="wt", bufs=1) as wt_pool,
            tc.tile_pool(name="xin", bufs=6) as in_pool,
            tc.tile_pool(name="yout", bufs=3) as out_pool,
            tc.tile_pool(name="mmps", bufs=4, space="PSUM") as ps_big,
            tc.tile_pool(name="nsps", bufs=4, space="PSUM") as ps_ns,
        ):
            ident = const_pool.tile([D, D], f32, tag="ident")
            make_identity(nc, ident)
            twoE = const_pool.tile([D, D], f32, tag="twoE")
            nc.vector.tensor_scalar_mul(twoE, ident, 2.0)

            # --- Newton-Schulz per head: WT = Ginv @ (I + S), G = (1+eps)I - S
            # bass matmul computes lhsT.T @ rhs; note A := (1+eps)I + S = G^T.
            wts = []
            for h in range(HPC):
                s_sb = const_pool.tile([D, D], f32, tag=f"s{h}")
                nc.sync.dma_start(out=s_sb, in_=s_d[h * D:(h + 1) * D, :])
                a_mat = const_pool.tile([D, D], f32, tag=f"amat{h}")
                nc.vector.scalar_tensor_tensor(
                    out=a_mat, in0=ident, scalar=1.0 + EPS, in1=s_sb,
                    op0=Alu.mult, op1=Alu.add)
                ips = const_pool.tile([D, D], f32, tag=f"ips{h}")
                nc.vector.tensor_add(ips, ident, s_sb)
                g_mat = const_pool.tile([D, D], f32, tag=f"g{h}")
                nc.vector.scalar_tensor_tensor(
                    out=g_mat, in0=ident, scalar=1.0 + EPS, in1=s_sb,
                    op0=Alu.mult, op1=Alu.subtract)

                X = ns_pool.tile([D, D], f32, tag=f"x{h}")
                nc.vector.tensor_scalar_mul(X, a_mat, NS_C)    # X0 = c G^T
                XT = ns_pool.tile([D, D], f32, tag=f"xt{h}")
                nc.vector.tensor_scalar_mul(XT, g_mat, NS_C)   # X0^T = c G

                for k in range(NS_ITERS):
                    t_ps = ps_ns.tile([D, D], f32, tag="nsps")
                    nc.tensor.matmul(t_ps, lhsT=a_mat, rhs=X, start=True, stop=True)  # G X
                    t2 = ns_pool.tile([D, D], f32, tag=f"t2{h}")
                    nc.vector.tensor_sub(t2, twoE, t_ps)       # 2I - G X
                    if k < NS_ITERS - 1:
                        xn_ps = ps_ns.tile([D, D], f32, tag="nsps")
                        nc.tensor.matmul(xn_ps, lhsT=XT, rhs=t2, start=True, stop=True)  # X T2
                        Xn = ns_pool.tile([D, D], f32, tag=f"x{h}")
                        nc.scalar.copy(Xn, xn_ps)
                        X = Xn
                    xtn_ps = ps_ns.tile([D, D], f32, tag="nsps")
                    nc.tensor.matmul(xtn_ps, lhsT=t2, rhs=XT, start=True, stop=True)  # (X T2)^T
                    XTn = ns_pool.tile([D, D], f32, tag=f"xt{h}")
                    nc.scalar.copy(XTn, xtn_ps)
                    XT = XTn

                wt_ps = ps_ns.tile([D, D], f32, tag="nsps")
                nc.tensor.matmul(wt_ps, lhsT=XT, rhs=ips, start=True, stop=True)  # Ginv (I+S)
                whi = wt_pool.tile([D, D], bf16, tag=f"whi{h}")
                nc.vector.tensor_copy(whi, wt_ps)
                wts.append(whi)

            # --- streaming panel matmul: yT[h] = W @ xT[h], all-bf16 I/O.
            # Single bf16 pass (PE at 1 cycle/row) with fp32 PSUM accum;
            # the kernel is HBM-bound so compute fully hides under DMA.
            half = CHUNK // 2
            for h in range(HPC):
                whi = wts[h]
                r0 = h * D
                for ci in range(T // CHUNK):
                    c0 = ci * CHUNK
                    xh = in_pool.tile([D, CHUNK], bf16, tag="xh")
                    nc.sync.dma_start(out=xh, in_=x_d[r0:r0 + D, c0:c0 + CHUNK])
                    yout = out_pool.tile([D, CHUNK], bf16, tag="yout")
                    for j in range(CHUNK // MM):
                        sl = slice(j * MM, (j + 1) * MM)
                        ps = ps_big.tile([D, MM], f32, tag="mm")
                        nc.tensor.matmul(ps, lhsT=whi, rhs=xh[:, sl],
                                         start=True, stop=True)
                        if j % 2 == 0:
                            nc.vector.tensor_copy(yout[:, sl], ps)
                        else:
                            nc.scalar.copy(yout[:, sl], ps)
                    # two half-stores so the DMA overlaps the second half's evac
                    nc.scalar.dma_start(out=yT_d[r0:r0 + D, c0:c0 + half],
                                        in_=yout[:, 0:half])
                    nc.scalar.dma_start(out=yT_d[r0:r0 + D, c0 + half:c0 + CHUNK],
                                        in_=yout[:, half:CHUNK])
    nc.compile()
    return nc


def _get_nc():
    if "nc" not in _CACHE:
        _CACHE["nc"] = _build_nc()
    return _CACHE["nc"]


def _prep_inputs(x, S_raw):
    """Host-side shard + layout prep. Returns per-core input maps."""
    import ml_dtypes

    bf16 = ml_dtypes.bfloat16
    x = np.asarray(x, dtype=np.float32)
    S_raw = np.asarray(S_raw, dtype=np.float32)
    S = S_raw - S_raw.transpose(0, 2, 1)
    # (B,H,N,D) -> (H, D, B*N), token-major per head; single bf16 tensor
    xT_full = np.ascontiguousarray(x.transpose(1, 3, 0, 2)).reshape(H * D, T)
    xbf = xT_full.astype(bf16)
    S_full = np.ascontiguousarray(S).reshape(H * D, D)
    in_maps = []
    for c in range(N_CORES):
        r = c * HPC * D
        in_maps.append({
            "x": xbf[r:r + HPC * D],
            "s": S_full[r:r + HPC * D],
        })
    return in_maps


def _postprocess(results):
    """Gather per-core yT shards back into (B, H, N, D) fp32."""
    yT_full = np.concatenate(
        [np.asarray(r["yT"], dtype=np.float32) for r in results], axis=0)
    y = yT_full.reshape(H, D, B, N).transpose(2, 0, 3, 1)
    return np.ascontiguousarray(y)


def _execute(in_maps, trace=False, **kwargs):
    _ensure_path()
    from concourse.bass_utils import run_bass_kernel_spmd

    nc = _get_nc()
    return run_bass_kernel_spmd(nc, in_maps, core_ids=list(range(N_CORES)),
                                trace=trace, **kwargs)


def kernel(x, S_raw):
    in_maps = _prep_inputs(x, S_raw)
    res = _execute(in_maps)
    return _postprocess(res.results)


# revision 10
# speedup vs baseline: 1.7694x; 1.2869x over previous
"""Cayley orthogonal transform kernel for Trainium2 (8 NeuronCores).

Math: per head h, y = (I - S) ((1+eps) I + S)^{-1} x applied along D=128,
where S = S_raw - S_raw^T is skew-symmetric.  With A = (1+eps)I + S and
G = A^T = (1+eps)I - S, the applied matrix satisfies
    W = (I - S) A^{-1} = (2+eps) A^{-1} - I,
so only A^{-1} = U G with U = (G G^T)^{-1} is needed.  U is computed with a
Newton-Schulz iteration in residual form on the SPD matrix P = G G^T:
    E <- E^2,  U <- U + U E        (E0 = I - cP, U0 = cI)
where everything commutes (polynomials in P), letting both products be one
128x256 matmul per iteration (U|E packed side by side) -- a short, mostly
off-critical-path preamble.

Pipeline (per core = 2 heads, tensor parallel over heads):
  * Host: skew-symmetrize S_raw, lay x out as xT[h, d, token] bf16.
  * Device: all input DMAs are issued up front into whole half-panel SBUF
    tiles (4 x 2 MiB, no buffer reuse -> no WAR stalls); NS runs under the
    input prefetch; then 512-column bf16 matmuls stream each panel through
    the PE array, PSUM is evacuated as bf16 alternating Vector/Scalar, and
    1 MiB output DMAs go back to DRAM on the ACT queue.
  * Both x and y travel as single bf16 tensors (~3e-3 rel error, well within
    tolerance): ~16.8 MB per core, i.e. the per-core HBM roofline
    (~358 GB/s) is the binding resource; all compute hides under the DMA.
  * Host: upcast to fp32 + inverse layout transform back to (B, H, N, D).
"""

import os
import sys

import numpy as np

B, H, N, D = 4, 16, 4096, 128
N_CORES = 8
HPC = H // N_CORES          # heads per core
T = B * N                   # tokens per head
PIECE = 8192                # columns per input DMA (2 MiB bf16)
OUT_PIECE = 4096            # columns per output DMA (1 MiB bf16)
MM = 512                    # columns per matmul (one PSUM bank)
NS_ITERS = 5                # Newton-Schulz iterations (residual 0.6^32 ~ 6e-8)
NS_C = 0.4                  # NS scale: safe for ||S||_2 < ~2 (actual ~1.72)
EPS = 1e-5

_CACHE = {}


def _ensure_path():
    for p in ("/opt/trn_rl_repo", "/root/.axon_site/_ro/trn_rl_repo"):
        if os.path.isdir(p) and p not in sys.path:
            sys.path.insert(0, p)
    _install_ntff_hook()


def _install_ntff_hook():
    """The agent image's ``antenv`` lacks ``axon_hooks``, which makes
    ``run_bass_kernel_spmd(trace=True)`` crash instead of degrading.  Provide
    the module and register the ctypes NTFF hook the boot shim would have."""
    if "antenv.axon_hooks" in sys.modules:
        return
    try:
        import types

        import antenv

        if hasattr(antenv, "axon_hooks"):
            return
        mod = types.ModuleType("antenv.axon_hooks")
        state = {"hook": None}
        mod.set_axon_ntff_profile_hook = lambda h: state.__setitem__("hook", h)
        mod.get_axon_ntff_profile_hook = lambda: state["hook"]
        sys.modules["antenv.axon_hooks"] = mod
        antenv.axon_hooks = mod
        try:
            from trn_agent_boot.trn_boot import _ntff_profile_via_ctypes

            so_path = "/opt/axon/libaxon_pjrt.so"
            if os.path.exists(so_path):
                mod.set_axon_ntff_profile_hook(_ntff_profile_via_ctypes(so_path))
        except Exception:
            pass  # hook stays None -> concourse logs + skips tracing
    except Exception:
        pass


def _build_nc():
    """Build the (single-program SPMD) Bass kernel for one core's shard."""
    _ensure_path()
    import concourse.tile as tile
    from concourse import bacc, mybir
    from concourse.masks import make_identity

    f32 = mybir.dt.float32
    bf16 = mybir.dt.bfloat16
    Alu = mybir.AluOpType

    nc = bacc.Bacc("TRN2", target_bir_lowering=False, debug=False)
    x_d = nc.dram_tensor("x", [HPC * D, T], bf16, kind="ExternalInput").ap()
    s_d = nc.dram_tensor("s", [HPC * D, D], f32, kind="ExternalInput").ap()
    yT_d = nc.dram_tensor("yT", [HPC * D, T], bf16, kind="ExternalOutput").ap()

    n_pieces = T // PIECE

    with tile.TileContext(nc) as tc:
        with (
            tc.tile_pool(name="const", bufs=1) as const_pool,
            tc.tile_pool(name="ns", bufs=4) as ns_pool,
            tc.tile_pool(name="xin", bufs=1) as in_pool,
            tc.tile_pool(name="yout", bufs=1) as out_pool,
            tc.tile_pool(name="psns", bufs=2, space="PSUM") as ps_ns,
            tc.tile_pool(name="psmm", bufs=6, space="PSUM") as ps_mm,
        ):
            # ---- all input DMAs first: S matrices, then every x half-panel.
            # Distinct destination tiles -> no WAR hazards, so the Sync queue
            # streams them back to back from t~0.
            s_sb = []
            for h in range(HPC):
                t_ = const_pool.tile([D, D], f32, tag=f"s{h}")
                nc.sync.dma_start(out=t_, in_=s_d[h * D:(h + 1) * D, :])
                s_sb.append(t_)
            xin = {}
            for h in range(HPC):
                for p in range(n_pieces):
                    t_ = in_pool.tile([D, PIECE], bf16, tag=f"x{h}_{p}",
                                      name=f"x{h}_{p}")
                    c0 = p * PIECE
                    nc.sync.dma_start(
                        out=t_, in_=x_d[h * D:(h + 1) * D, c0:c0 + PIECE])
                    xin[(h, p)] = t_

            ident = const_pool.tile([D, D], f32, tag="ident")
            make_identity(nc, ident)

            # ---- Newton-Schulz (residual form), heads interleaved so the two
            # dependency chains pipeline on the engines.
            a_mat, g_mat, ue = [], [], []
            for h in range(HPC):
                a_ = const_pool.tile([D, D], f32, tag=f"amat{h}")
                nc.vector.scalar_tensor_tensor(
                    out=a_, in0=ident, scalar=1.0 + EPS, in1=s_sb[h],
                    op0=Alu.mult, op1=Alu.add)          # A = (1+eps)I + S
                a_mat.append(a_)
                g_ = const_pool.tile([D, D], f32, tag=f"gmat{h}")
                nc.vector.scalar_tensor_tensor(
                    out=g_, in0=ident, scalar=1.0 + EPS, in1=s_sb[h],
                    op0=Alu.mult, op1=Alu.subtract)     # G = A^T
                g_mat.append(g_)

            p0 = []
            for h in range(HPC):
                ps = ps_ns.tile([D, D], f32, tag="nsps")
                # lhsT=A -> A^T A = G G^T = P
                nc.tensor.matmul(ps, lhsT=a_mat[h], rhs=a_mat[h],
                                 start=True, stop=True)
                p0.append(ps)
            for h in range(HPC):
                ue0 = ns_pool.tile([D, 2 * D], f32, tag=f"ue{h}")
                nc.vector.tensor_scalar_mul(ue0[:, 0:D], ident, NS_C)  # U0 = cI
                nc.vector.scalar_tensor_tensor(
                    out=ue0[:, D:2 * D], in0=p0[h], scalar=-NS_C, in1=ident,
                    op0=Alu.mult, op1=Alu.add)          # E0 = I - cP
                ue.append(ue0)

            for k in range(NS_ITERS):
                prods = []
                for h in range(HPC):
                    ps = ps_ns.tile([D, 2 * D], f32, tag="nsps")
                    # lhsT = E (symmetric): [E U | E E] = [U E | E^2]
                    nc.tensor.matmul(ps, lhsT=ue[h][:, D:2 * D], rhs=ue[h],
                                     start=True, stop=True)
                    prods.append(ps)
                for h in range(HPC):
                    nxt = ns_pool.tile([D, 2 * D], f32, tag=f"ue{h}")
                    nc.vector.tensor_add(nxt[:, 0:D], ue[h][:, 0:D],
                                         prods[h][:, 0:D])      # U += U E
                    nc.scalar.copy(nxt[:, D:2 * D], prods[h][:, D:2 * D])
                    ue[h] = nxt

            wts = []
            for h in range(HPC):
                ps = ps_ns.tile([D, D], f32, tag="nsps")
                # lhsT=G -> G^T U = A U = G^{-1}  (G^{-1} = G^T P^{-1})
                nc.tensor.matmul(ps, lhsT=g_mat[h], rhs=ue[h][:, 0:D],
                                 start=True, stop=True)
                w_ = const_pool.tile([D, D], bf16, tag=f"w{h}")
                nc.vector.scalar_tensor_tensor(
                    out=w_, in0=ps, scalar=2.0 + EPS, in1=ident,
                    op0=Alu.mult, op1=Alu.subtract)  # W^T = (2+eps) A U - I
                wts.append(w_)

            # ---- streaming panel matmuls: yT[h] = W @ xT[h] (all-bf16 I/O)
            for h in range(HPC):
                yo = {}
                for p in range(n_pieces):
                    yo[p] = out_pool.tile([D, PIECE], bf16, tag=f"y{h}_{p}",
                                          name=f"y{h}_{p}")
                for p in range(n_pieces):
                    xt, yt = xin[(h, p)], yo[p]
                    for j in range(PIECE // MM):
                        sl = slice(j * MM, (j + 1) * MM)
                        ps = ps_mm.tile([D, MM], f32, tag="mm")
                        nc.tensor.matmul(ps, lhsT=wts[h], rhs=xt[:, sl],
                                         start=True, stop=True)
                        if j % 2 == 0:
                            nc.vector.tensor_copy(yt[:, sl], ps)
                        else:
                            nc.scalar.copy(yt[:, sl], ps)
                        col_end = (j + 1) * MM
                        if col_end % OUT_PIECE == 0:
                            o0 = col_end - OUT_PIECE
                            is_last = (h == HPC - 1 and p == n_pieces - 1
                                       and col_end == PIECE)
                            g0 = p * PIECE + o0
                            if is_last:
                                # split the final store so the drain tail is
                                # one 512 KiB DMA, not 1 MiB
                                hp = OUT_PIECE // 2
                                nc.scalar.dma_start(
                                    out=yT_d[h * D:(h + 1) * D, g0:g0 + hp],
                                    in_=yt[:, o0:o0 + hp])
                                nc.scalar.dma_start(
                                    out=yT_d[h * D:(h + 1) * D,
                                             g0 + hp:g0 + OUT_PIECE],
                                    in_=yt[:, o0 + hp:o0 + OUT_PIECE])
                            else:
                                nc.scalar.dma_start(
                                    out=yT_d[h * D:(h + 1) * D,
                                             g0:g0 + OUT_PIECE],
                                    in_=yt[:, o0:o0 + OUT_PIECE])
    nc.compile()
    return nc


def _get_nc():
    if "nc" not in _CACHE:
        _CACHE["nc"] = _build_nc()
    return _CACHE["nc"]


def _prep_inputs(x, S_raw):
    """Host-side shard + layout prep. Returns per-core input maps."""
    import ml_dtypes

    bf16 = ml_dtypes.bfloat16
    x = np.asarray(x, dtype=np.float32)
    S_raw = np.asarray(S_raw, dtype=np.float32)
    S = S_raw - S_raw.transpose(0, 2, 1)
    # (B,H,N,D) -> (H, D, B*N), token-major per head; single bf16 tensor
    xT_full = np.ascontiguousarray(x.transpose(1, 3, 0, 2)).reshape(H * D, T)
    xbf = xT_full.astype(bf16)
    S_full = np.ascontiguousarray(S).reshape(H * D, D)
    in_maps = []
    for c in range(N_CORES):
        r = c * HPC * D
        in_maps.append({
            "x": xbf[r:r + HPC * D],
            "s": S_full[r:r + HPC * D],
        })
    return in_maps


def _postprocess(results):
    """Gather per-core yT shards back into (B, H, N, D) fp32."""
    yT_full = np.concatenate(
        [np.asarray(r["yT"], dtype=np.float32) for r in results], axis=0)
    y = yT_full.reshape(H, D, B, N).transpose(2, 0, 3, 1)
    return np.ascontiguousarray(y)


def _execute(in_maps, trace=False, **kwargs):
    _ensure_path()
    from concourse.bass_utils import run_bass_kernel_spmd

    nc = _get_nc()
    return run_bass_kernel_spmd(nc, in_maps, core_ids=list(range(N_CORES)),
                                trace=trace, **kwargs)


def kernel(x, S_raw):
    in_maps = _prep_inputs(x, S_raw)
    res = _execute(in_maps)
    return _postprocess(res.results)
